# revision 16
# baseline (speedup 1.0000x reference)
"""Dark-Channel-Prior dehazing (DCPGenerator) Trainium2 Bass kernel.

Contract: kernel(x: [16,3,512,512] f32) -> [16,3,512,512] f32.
Data-parallel over 8 NeuronCores: 2 samples per core. Each core runs the
full per-sample pipeline on-device:
  guidance/img prep -> dark channel (15x15 min-pool) -> atmospheric light
  (top-1% selection via secant-estimated threshold + band-corrected mean)
  -> second dark channel on img/A (bf16) -> guided filter (r=40 box sums
  via free-dim scans + PE triangular-matmul partition cumsums) -> output.
"""
import numpy as np
from contextlib import ExitStack

H = 512
W = 512
NCHUNK = 4          # 4 row-chunks of 128 partitions
CW = 512            # chunk free width
PADW = 526          # padded chunk width for the 15-wide min pool (7+512+7)
WIN_PAD = 7
RADIUS = 40
EPS = 1e-3
OMEGA = 0.95
TOPN = int(0.01 * H * W)          # 2621
T0 = 0.0055                       # secant bracket on raw-x dark scale
T1 = 0.0085
BAND = 2e-4                       # band width for tie-region correction
SECANT_ROUNDS = 6

_CACHE = {}
_DEBUG_TAPS = False


# ---------------------------------------------------------------- host consts
def _host_consts():
    n1 = np.minimum(np.arange(H) + RADIUS, H - 1) - np.maximum(np.arange(H) - RADIUS, 0) + 1
    inv_nh = (1.0 / n1).astype(np.float32)          # [512]
    inv_nw = inv_nh.copy()                          # same for W=512
    invnh = np.zeros((128, NCHUNK), np.float32)
    for c in range(NCHUNK):
        invnh[:, c] = inv_nh[c * 128:(c + 1) * 128]
    invnw_rep = np.broadcast_to(inv_nw[None, :], (128, W)).copy()
    tri = np.triu(np.ones((128, 128), np.float32)).copy()   # lhsT[k,p]=1 iff k<=p
    return {"tri": tri, "invnh": invnh, "invnw": invnw_rep}


# ------------------------------------------------------------------ program
def _build():
    import concourse.bacc as bacc
    import concourse.tile as tile
    import concourse.bass as bass
    from concourse import mybir

    f32 = mybir.dt.float32
    bf16 = mybir.dt.bfloat16
    Alu = mybir.AluOpType
    Act = mybir.ActivationFunctionType

    nc = bacc.Bacc("TRN2", target_bir_lowering=False, debug=False, num_devices=8)

    x_ext = nc.dram_tensor("x", [2, 3, H, W], f32, kind="ExternalInput").ap()
    tri_ext = nc.dram_tensor("tri", [128, 128], f32, kind="ExternalInput").ap()
    invnh_ext = nc.dram_tensor("invnh", [128, NCHUNK], f32, kind="ExternalInput").ap()
    invnw_ext = nc.dram_tensor("invnw", [128, W], f32, kind="ExternalInput").ap()
    y_ext = nc.dram_tensor("y", [2, 3, H, W], f32, kind="ExternalOutput").ap()
    dbg = {}
    if _DEBUG_TAPS:
        for nm in ("u", "guid", "p", "hbI", "mI", "mp", "a", "b", "T", "rT"):
            dbg[nm] = nc.dram_tensor(f"dbg_{nm}", [128, NCHUNK * CW], f32,
                                     kind="ExternalOutput").ap()
        dbg["dark2"] = nc.dram_tensor("dbg_dark2", [128, NCHUNK * CW], bf16,
                                      kind="ExternalOutput").ap()
        dbg["tot"] = nc.dram_tensor("dbg_tot", [1, 8], f32, kind="ExternalOutput").ap()
        dbg["am"] = nc.dram_tensor("dbg_am", [1, 12], f32, kind="ExternalOutput").ap()
        dbg["chsc"] = nc.dram_tensor("dbg_chsc", [128, 9], f32, kind="ExternalOutput").ap()
        dbg["scal0"] = nc.dram_tensor("dbg_scal0", [1, 16], f32, kind="ExternalOutput").ap()
        dbg["thr0"] = nc.dram_tensor("dbg_thr0", [128, 1], f32, kind="ExternalOutput").ap()
        dbg["acc0"] = nc.dram_tensor("dbg_acc0", [128, 8], f32, kind="ExternalOutput").ap()
        dbg["uh"] = nc.dram_tensor("dbg_uh", [128, NCHUNK * CW], f32, kind="ExternalOutput").ap()
        dbg["scalr"] = nc.dram_tensor("dbg_scalr", [8, 16], f32, kind="ExternalOutput").ap()

    def cview(t, width=CW):
        """[128, NCHUNK*width] tile -> [128, NCHUNK, width] view."""
        return t.rearrange("p (c w) -> p c w", w=width)

    def fbcast(ap_col, n):
        """free-dim step-0 broadcast of a [...,1] AP to [...,n]."""
        return bass.AP(tensor=ap_col.tensor, offset=ap_col.offset,
                       ap=[list(p) for p in ap_col.ap[:-1]] + [[0, n]])

    with ExitStack() as ctx:
        tc = ctx.enter_context(tile.TileContext(nc))

        cpool = ctx.enter_context(tc.tile_pool(name="cpool", bufs=1))
        big = ctx.enter_context(tc.tile_pool(name="big", bufs=1))
        pp = ctx.enter_context(tc.tile_pool(name="pp", bufs=1))       # minpool / box scratch
        boxes = ctx.enter_context(tc.tile_pool(name="boxes", bufs=5))
        srcp = ctx.enter_context(tc.tile_pool(name="srcp", bufs=3))
        abt = ctx.enter_context(tc.tile_pool(name="abt", bufs=3))
        tiny = ctx.enter_context(tc.tile_pool(name="tiny", bufs=2))
        pbig = ctx.enter_context(tc.tile_pool(name="pbig", bufs=2, space="PSUM"))
        psml = ctx.enter_context(tc.tile_pool(name="psml", bufs=2, space="PSUM"))

        # ---- constants ----
        c_tri = cpool.tile([128, 128], f32, name="c_tri")
        nc.sync.dma_start(out=c_tri[:], in_=tri_ext[:])
        c_invnh = cpool.tile([128, NCHUNK], f32, name="c_invnh")
        nc.sync.dma_start(out=c_invnh[:], in_=invnh_ext[:])
        c_invnw = cpool.tile([128, W], f32, name="c_invnw")
        nc.sync.dma_start(out=c_invnw[:], in_=invnw_ext[:])
        c_ones128 = cpool.tile([128, 1], f32, name="c_ones128")
        nc.vector.memset(c_ones128[:], 1.0)
        c_ones1x = cpool.tile([1, 128], f32, name="c_ones1x")
        nc.vector.memset(c_ones1x[:], 1.0)
        c_zeros = cpool.tile([128, CW], f32, name="c_zeros")
        nc.vector.memset(c_zeros[:], 0.0)
        c_ones = cpool.tile([128, CW], f32, name="c_ones")
        nc.vector.memset(c_ones[:], 1.0)
        c_ones16 = cpool.tile([128, CW], bf16, name="c_ones16")
        nc.vector.memset(c_ones16[:], 1.0)

        # ------------------------------------------------ helpers (emit ops)
        def memset_pads(t, dt, eng):
            v = cview(t, PADW)
            for c in range(NCHUNK):
                eng.memset(v[:, c, 0:WIN_PAD], 1.0)
                eng.memset(v[:, c, PADW - WIN_PAD:PADW], 1.0)

        def hpool(dst, padded, w1, eng):
            """15-wide sliding min along free dim; padded [128,4*526] -> dst [128,4*512]."""
            a = cview(padded, PADW)
            b = cview(w1, PADW)
            d = cview(dst)
            for c in range(NCHUNK):
                eng.tensor_tensor(b[:, c, 0:525], a[:, c, 0:525], a[:, c, 1:526], Alu.min)
            for c in range(NCHUNK):
                eng.tensor_tensor(a[:, c, 0:523], b[:, c, 0:523], b[:, c, 2:525], Alu.min)
            for c in range(NCHUNK):
                eng.tensor_tensor(b[:, c, 0:519], a[:, c, 0:519], a[:, c, 4:523], Alu.min)
            for c in range(NCHUNK):
                eng.tensor_tensor(d[:, c, :], b[:, c, 0:512], b[:, c, 7:519], Alu.min)

        def vshift_dma(dst, src, s, pad_tile):
            """dst[row r] = src[row r+s] (global 512-row space), bottom s rows from pad."""
            dv, sv = cview(dst), cview(src)
            for c in range(NCHUNK):
                nc.sync.dma_start(out=dv[0:128 - s, c, :], in_=sv[s:128, c, :])
                if c < NCHUNK - 1:
                    nc.sync.dma_start(out=dv[128 - s:128, c, :], in_=sv[0:s, c + 1, :])
                else:
                    nc.sync.dma_start(out=dv[128 - s:128, c, :], in_=pad_tile[0:s, :])

        def vshift_dma_down(dst, src, s, pad_tile):
            """dst[row r] = src[max(r-s, 0)] (clamped at the top edge)."""
            dv, sv = cview(dst), cview(src)
            for c in range(NCHUNK):
                nc.sync.dma_start(out=dv[s:128, c, :], in_=sv[0:128 - s, c, :])
                if c > 0:
                    nc.sync.dma_start(out=dv[0:s, c, :], in_=sv[128 - s:128, c - 1, :])
                else:
                    for k in range(s):
                        nc.sync.dma_start(out=dv[k:k + 1, 0, :], in_=sv[0:1, 0, :])

        def hbox(dst, src, cum, eng):
            """zero-padded 81-wide box sum along free dim. src,dst,cum [128,2048]."""
            sv, dv, cv = cview(src), cview(dst), cview(cum)
            for c in range(NCHUNK):
                eng.tensor_tensor_scan(cv[:, c, :], sv[:, c, :], c_zeros[:],
                                       0.0, Alu.add, Alu.add)
            for c in range(NCHUNK):
                eng.tensor_copy(dv[:, c, 0:41], cv[:, c, 40:81])
                eng.tensor_tensor(dv[:, c, 41:472], cv[:, c, 81:512], cv[:, c, 0:431],
                                  Alu.subtract)
                eng.tensor_tensor(dv[:, c, 472:512], fbcast(cv[:, c, 511:512], 40),
                                  cv[:, c, 431:471], Alu.subtract)

        def vbox(dst, src, vcum, hi, lo, eng, scale_invn=True):
            """zero-padded 81-tall box sum down partition rows (global 512-row space).
            src -> dst [128,2048]; vcum/hi/lo scratch [128,2048]."""
            sv, vv = cview(src), cview(vcum)
            carry = [None] * NCHUNK
            cps = psml.tile([1, CW], f32, name="carry_ps", tag="carry_ps")
            for c in range(NCHUNK - 1):
                nc.tensor.matmul(cps[:], c_ones128[:], sv[:, c, :],
                                 start=(c == 0), stop=True, skip_group_check=True)
                car = tiny.tile([1, CW], f32, name=f"car{c}", tag=f"car{c}")
                nc.scalar.copy(car[:], cps[:])
                carry[c + 1] = car
            for c in range(NCHUNK):
                vps = pbig.tile([128, CW], f32, name="vps", tag="vps")
                nc.tensor.matmul(vps[:], c_tri[:], sv[:, c, :], start=True,
                                 stop=(carry[c] is None))
                if carry[c] is not None:
                    nc.tensor.matmul(vps[:], c_ones1x[:], carry[c][:],
                                     start=False, stop=True)
                nc.scalar.copy(vv[:, c, :], vps[:])
            # hi[r] = vcum[min(r+40, 511)]
            hv, lv = cview(hi), cview(lo)
            for c in range(NCHUNK):
                nc.sync.dma_start(out=hv[0:88, c, :], in_=vv[40:128, c, :])
                if c < NCHUNK - 1:
                    nc.sync.dma_start(out=hv[88:128, c, :], in_=vv[0:40, c + 1, :])
            rl = tiny.tile([1, CW], f32, name="rowlast", tag="rowlast")
            nc.sync.dma_start(out=rl[:], in_=vv[127:128, NCHUNK - 1, :])
            bps = psml.tile([40, CW], f32, name="bcast_ps", tag="carry_ps")
            nc.tensor.matmul(bps[:], c_ones1x[0:1, 0:40], rl[:], start=True, stop=True)
            bch = tiny.tile([40, CW], f32, name="bch", tag="bch", bufs=1)
            nc.scalar.copy(bch[:], bps[:])
            nc.sync.dma_start(out=hv[88:128, NCHUNK - 1, :], in_=bch[0:40, :])
            # lo[r] = r >= 41 ? vcum[r-41] : 0
            nc.sync.dma_start(out=lv[41:128, 0, :], in_=vv[0:87, 0, :])
            nc.sync.dma_start(out=lv[0:41, 0, :], in_=c_zeros[0:41, :])
            for c in range(1, NCHUNK):
                nc.sync.dma_start(out=lv[0:41, c, :], in_=vv[87:128, c - 1, :])
                nc.sync.dma_start(out=lv[41:128, c, :], in_=vv[0:87, c, :])
            # dst = (hi - lo) * invnh * invnw   (or raw box sum if not scale_invn)
            eng.tensor_tensor(hi[:], hi[:], lo[:], Alu.subtract)
            dv = cview(dst)
            hvv = cview(hi)
            if scale_invn:
                for c in range(NCHUNK):
                    eng.scalar_tensor_tensor(dv[:, c, :], hvv[:, c, :],
                                             c_invnh[:, c:c + 1], c_invnw[:],
                                             Alu.mult, Alu.mult)
            else:
                eng.tensor_copy(dst[:], hi[:])

        # ======================================================== per sample
        for s in range(2):
            V = nc.vector
            G = nc.vector  # walrus rejects elementwise ops on the Pool engine

            def tap(nm, t):
                if _DEBUG_TAPS and s == 0:
                    nc.sync.dma_start(out=dbg[nm][:], in_=t[:])

            # ---- load ----
            xch = []
            for chn in range(3):
                t = big.tile([128, NCHUNK * CW], f32, name=f"x{chn}", tag=f"x{chn}")
                for c in range(NCHUNK):
                    nc.sync.dma_start(out=cview(t)[:, c, :],
                                      in_=x_ext[s, chn, c * 128:(c + 1) * 128, :])
                xch.append(t)
            xr, xg, xb = xch

            # ---- guidance I = ((.2989 xr + .587 xg + .114 xb) + 1)/2  (gpsimd+ACT) ----
            Ia = pp.tile([128, NCHUNK * CW], f32, name="Ia", tag="sh")
            Ib = pp.tile([128, NCHUNK * CW], f32, name="Ib", tag="Ib")
            nc.scalar.activation(Ib[:], xr[:], Act.Copy, bias=0.5, scale=0.14945)
            G.scalar_tensor_tensor(Ia[:], xg[:], 0.2935, Ib[:], Alu.mult, Alu.add)
            G.scalar_tensor_tensor(Ib[:], xb[:], 0.057, Ia[:], Alu.mult, Alu.add)
            guid = Ib  # final guidance lives in Ib; Ia free for reuse
            tap("guid", guid)

            # ---- dark1 = minpool15(min_c x) on raw x (f32, exact) ----
            mxp = pp.tile([128, NCHUNK * PADW], f32, name="mxp", tag="mxp")
            w1 = pp.tile([128, NCHUNK * PADW], f32, name="w1", tag="w1")
            memset_pads(mxp, f32, V)
            mv = cview(mxp, PADW)
            for c in range(NCHUNK):
                V.tensor_tensor(mv[:, c, WIN_PAD:WIN_PAD + CW], cview(xr)[:, c, :],
                                cview(xg)[:, c, :], Alu.min)
                V.tensor_tensor(mv[:, c, WIN_PAD:WIN_PAD + CW],
                                mv[:, c, WIN_PAD:WIN_PAD + CW],
                                cview(xb)[:, c, :], Alu.min)
            uh = pp.tile([128, NCHUNK * CW], f32, name="uh", tag="uh")
            hpool(uh, mxp, w1, V)
            tap("uh", uh)
            sh = pp.tile([128, NCHUNK * CW], f32, name="sh", tag="sh")
            u2 = pp.tile([128, NCHUNK * CW], f32, name="u2", tag="u2")
            # vpool inline (explicit ping-pong)
            vshift_dma(sh, uh, 1, c_ones)
            V.tensor_tensor(u2[:], uh[:], sh[:], Alu.min)
            vshift_dma(sh, u2, 2, c_ones)
            V.tensor_tensor(uh[:], u2[:], sh[:], Alu.min)
            vshift_dma(sh, uh, 4, c_ones)
            V.tensor_tensor(u2[:], uh[:], sh[:], Alu.min)
            vshift_dma_down(sh, u2, 7, c_ones)
            u = uh
            V.tensor_tensor(u[:], u2[:], sh[:], Alu.min)
            tap("u", u)

            # ---- atmospheric light ----
            junk = pp.tile([128, NCHUNK * CW], f32, name="junk", tag="mxp")
            acc8 = tiny.tile([128, 8], f32, name="acc8", tag="acc8")
            V.memset(acc8[:], 0.0)
            thr = tiny.tile([128, 1], f32, name="thr", tag="thr")
            scal = tiny.tile([1, 16], f32, name="scal", tag="scal")
            V.memset(scal[:], 0.0)
            # scal cols: 0 ta, 1 Ca, 2 tb, 3 Cb, 4..temp
            V.memset(scal[:, 0:1], T0)
            V.memset(scal[:, 2:3], T1)

            def count_into(col):
                V.tensor_scalar(junk[:], u[:], thr[:], 0.0, Alu.is_gt, Alu.add,
                                accum_out=acc8[:, col:col + 1])
                fps = psml.tile([1, 1], f32, name="fold_ps", tag="fold_ps")
                nc.tensor.matmul(fps[:], c_ones128[:], acc8[:, col:col + 1],
                                 start=True, stop=True)
                return fps

            def bcast_thr(src_col):
                bp = psml.tile([128, 1], f32, name="thr_ps", tag="fold_ps")
                nc.tensor.matmul(bp[:], c_ones1x[:], src_col, start=True, stop=True)
                nc.scalar.copy(thr[:], bp[:])

            # C(t0), C(t1)
            bcast_thr(scal[0:1, 0:1])
            tap("thr0", thr)
            f = count_into(0)
            nc.scalar.copy(scal[:, 1:2], f[:])
            tap("acc0", acc8)
            tap("scal0", scal)
            bcast_thr(scal[0:1, 2:3])
            f = count_into(0)
            nc.scalar.copy(scal[:, 3:4], f[:])
            for _rnd in range(SECANT_ROUNDS):
                # count is monotone non-increasing in t, so sign(dC) = -sign(dT);
                # step = (R - Cb) * dT/dC = (Cb - R) * |dT| / max(|dC|, 1)
                V.tensor_tensor(scal[:, 4:5], scal[:, 3:4], scal[:, 1:2], Alu.subtract)
                V.tensor_scalar(scal[:, 8:9], scal[:, 4:5], -1.0, 0.0, Alu.mult, Alu.add)
                V.tensor_tensor(scal[:, 4:5], scal[:, 4:5], scal[:, 8:9], Alu.max)
                V.tensor_scalar(scal[:, 4:5], scal[:, 4:5], 1.0, 0.0, Alu.max, Alu.add)
                V.tensor_tensor(scal[:, 5:6], scal[:, 2:3], scal[:, 0:1], Alu.subtract)
                V.tensor_scalar(scal[:, 8:9], scal[:, 5:6], -1.0, 0.0, Alu.mult, Alu.add)
                V.tensor_tensor(scal[:, 5:6], scal[:, 5:6], scal[:, 8:9], Alu.max)
                V.reciprocal(scal[:, 8:9], scal[:, 4:5])
                V.tensor_tensor(scal[:, 5:6], scal[:, 5:6], scal[:, 8:9], Alu.mult)
                V.tensor_scalar(scal[:, 6:7], scal[:, 3:4], 1.0, -float(TOPN),
                                Alu.mult, Alu.add)
                V.tensor_tensor(scal[:, 6:7], scal[:, 6:7], scal[:, 5:6], Alu.mult)
                V.tensor_copy(scal[:, 0:1], scal[:, 2:3])
                V.tensor_copy(scal[:, 1:2], scal[:, 3:4])
                V.tensor_tensor(scal[:, 2:3], scal[:, 2:3], scal[:, 6:7], Alu.add)
                bcast_thr(scal[0:1, 2:3])
                f = count_into(0)
                nc.scalar.copy(scal[:, 3:4], f[:])
                if _DEBUG_TAPS and s == 0:
                    nc.sync.dma_start(out=dbg["scalr"][_rnd:_rnd + 1, :], in_=scal[:])
            # final: C* already in acc8 col0 / scal3. masked sums at thr.
            for chn, xt in enumerate((xr, xg, xb)):
                V.scalar_tensor_tensor(junk[:], u[:], thr[:], xt[:], Alu.is_gt,
                                       Alu.mult, accum_out=acc8[:, 1 + chn:2 + chn])
            # band threshold = thr - BAND
            V.tensor_scalar(scal[:, 7:8], scal[:, 2:3], 1.0, -BAND, Alu.mult, Alu.add)
            bcast_thr(scal[0:1, 7:8])
            V.tensor_scalar(junk[:], u[:], thr[:], 0.0, Alu.is_gt, Alu.add,
                            accum_out=acc8[:, 4:5])
            for chn, xt in enumerate((xr, xg, xb)):
                V.scalar_tensor_tensor(junk[:], u[:], thr[:], xt[:], Alu.is_gt,
                                       Alu.mult, accum_out=acc8[:, 5 + chn:6 + chn])
            tps = psml.tile([1, 8], f32, name="tot_ps", tag="fold_ps")
            nc.tensor.matmul(tps[:], c_ones128[:], acc8[:], start=True, stop=True)
            tot = tiny.tile([1, 8], f32, name="tot", tag="tot")
            nc.scalar.copy(tot[:], tps[:])
            tap("tot", tot)
            # A math: tot = [C*, Sr, Sg, Sb, Cb, Sbr, Sbg, Sbb]
            am = tiny.tile([1, 12], f32, name="am", tag="am")
            # am cols: 0:3 A_img, 3:6 recipA2, 6:9 bias_d (0.5-A), 9 amt, 10 recdc, 11 tmp
            V.tensor_tensor(am[:, 0:3], tot[:, 5:8], tot[:, 1:4], Alu.subtract)  # dS
            V.tensor_tensor(am[:, 11:12], tot[:, 4:5], tot[:, 0:1], Alu.subtract)  # dC
            V.tensor_scalar(am[:, 11:12], am[:, 11:12], 1.0, 0.0, Alu.max, Alu.add)
            V.reciprocal(am[:, 10:11], am[:, 11:12])
            V.tensor_tensor(am[:, 0:3], am[:, 0:3], fbcast(am[:, 10:11], 3), Alu.mult)  # mu
            V.tensor_scalar(am[:, 9:10], tot[:, 0:1], -1.0, float(TOPN), Alu.mult, Alu.add)
            V.tensor_tensor(am[:, 0:3], am[:, 0:3], fbcast(am[:, 9:10], 3), Alu.mult)
            V.tensor_tensor(am[:, 0:3], am[:, 0:3], tot[:, 1:4], Alu.add)  # S + amt*mu
            V.tensor_scalar(am[:, 0:3], am[:, 0:3], 1.0 / TOPN, 0.0, Alu.mult, Alu.add)  # Ax
            V.tensor_scalar(am[:, 3:6], am[:, 0:3], 1.0, 1.0, Alu.mult, Alu.add)  # Ax+1
            V.reciprocal(am[:, 3:6], am[:, 3:6])                      # 1/(Ax+1) = 1/(2A)
            V.tensor_scalar(am[:, 0:3], am[:, 0:3], 0.5, 0.5, Alu.mult, Alu.add)  # A img
            V.tensor_scalar(am[:, 6:9], am[:, 0:3], -1.0, 0.5, Alu.mult, Alu.add)  # .5-A
            # broadcast per-channel scalars to [128,1]
            chsc = tiny.tile([128, 9], f32, name="chsc", tag="chsc")
            for k in range(9):
                bp = psml.tile([128, 1], f32, name="ch_ps", tag="fold_ps")
                nc.tensor.matmul(bp[:], c_ones1x[:], am[0:1, k:k + 1], start=True, stop=True)
                nc.scalar.copy(chsc[:, k:k + 1], bp[:])
            # cols 0:3 A_img, 3:6 recipA2, 6:9 bias_d
            tap("am", am)
            tap("chsc", chsc)

            # ---- dark2 (bf16) + p ----
            mxp16 = pp.tile([128, NCHUNK * PADW], bf16, name="mxp16", tag="mxp")
            w116 = pp.tile([128, NCHUNK * PADW], bf16, name="w116", tag="w1")
            memset_pads(mxp16, bf16, V)
            yr = srcp.tile([128, NCHUNK * CW], bf16, name="yr", tag="srcp")
            yg = srcp.tile([128, NCHUNK * CW], bf16, name="yg", tag="srcp")
            yb_ = srcp.tile([128, NCHUNK * CW], bf16, name="yb", tag="srcp")
            for yt, xt, k in ((yr, xr, 3), (yg, xg, 4), (yb_, xb, 5)):
                nc.scalar.activation(yt[:], xt[:], Act.Identity,
                                     bias=chsc[:, k:k + 1], scale=chsc[:, k:k + 1])
            m16 = cview(mxp16, PADW)
            for c in range(NCHUNK):
                V.tensor_tensor(m16[:, c, WIN_PAD:WIN_PAD + CW], cview(yr)[:, c, :],
                                cview(yg)[:, c, :], Alu.min)
                V.tensor_tensor(m16[:, c, WIN_PAD:WIN_PAD + CW],
                                m16[:, c, WIN_PAD:WIN_PAD + CW],
                                cview(yb_)[:, c, :], Alu.min)
            uh16 = pp.tile([128, NCHUNK * CW], bf16, name="uh16", tag="uh")
            hpool(uh16, mxp16, w116, V)
            sh16 = pp.tile([128, NCHUNK * CW], bf16, name="sh16", tag="sh")
            u216 = pp.tile([128, NCHUNK * CW], bf16, name="u216", tag="u2")
            vshift_dma(sh16, uh16, 1, c_ones16)
            V.tensor_tensor(u216[:], uh16[:], sh16[:], Alu.min)
            vshift_dma(sh16, u216, 2, c_ones16)
            V.tensor_tensor(uh16[:], u216[:], sh16[:], Alu.min)
            vshift_dma(sh16, uh16, 4, c_ones16)
            V.tensor_tensor(u216[:], uh16[:], sh16[:], Alu.min)
            vshift_dma_down(sh16, u216, 7, c_ones16)
            V.tensor_tensor(uh16[:], u216[:], sh16[:], Alu.min)
            tap("dark2", uh16)
            p = srcp.tile([128, NCHUNK * CW], f32, name="p", tag="srcp")
            nc.scalar.activation(p[:], uh16[:], Act.Identity, bias=1.0, scale=-OMEGA)
            tap("p", p)

            # ---- guided filter ----
            Ip = srcp.tile([128, NCHUNK * CW], f32, name="Ip", tag="srcp")
            V.tensor_tensor(Ip[:], guid[:], p[:], Alu.mult)
            II = srcp.tile([128, NCHUNK * CW], f32, name="II", tag="srcp")
            nc.scalar.activation(II[:], guid[:], Act.Square)

            cum = pp.tile([128, NCHUNK * CW], f32, name="cum", tag="u2")
            vcum = pp.tile([128, NCHUNK * CW], f32, name="vcum", tag="mxp")
            hi_t = pp.tile([128, NCHUNK * CW], f32, name="hi_t", tag="w1")
            lo_t = pp.tile([128, NCHUNK * CW], f32, name="lo_t", tag="sh")

            hbs = {}
            for nm, src_t, eng in (("I", guid, V), ("p", p, G), ("Ip", Ip, V), ("II", II, G)):
                hb_t = boxes.tile([128, NCHUNK * CW], f32, name=f"hb{nm}", tag="boxes")
                hbox(hb_t, src_t, cum, eng)
                hbs[nm] = hb_t
                if nm == "I":
                    tap("hbI", hb_t)
            means = {}
            for nm, eng in (("I", V), ("p", G), ("Ip", V), ("II", G)):
                mn = boxes.tile([128, NCHUNK * CW], f32, name=f"mean{nm}", tag="boxes")
                vbox(mn, hbs[nm], vcum, hi_t, lo_t, eng)
                means[nm] = mn
                if nm == "I":
                    tap("mI", mn)
                if nm == "p":
                    tap("mp", mn)
            mI, mp_, mIp, mII = means["I"], means["p"], means["Ip"], means["II"]

            tmp = abt.tile([128, NCHUNK * CW], f32, name="tmp", tag="abt")
            V.tensor_tensor(tmp[:], mI[:], mp_[:], Alu.mult)
            cov = abt.tile([128, NCHUNK * CW], f32, name="cov", tag="abt")
            V.tensor_tensor(cov[:], mIp[:], tmp[:], Alu.subtract)
            sq = abt.tile([128, NCHUNK * CW], f32, name="sq", tag="abt")
            nc.scalar.activation(sq[:], mI[:], Act.Square)
            G.tensor_scalar(sq[:], sq[:], -1.0, EPS, Alu.mult, Alu.add)
            G.tensor_tensor(sq[:], sq[:], mII[:], Alu.add)       # var + eps
            rec = abt.tile([128, NCHUNK * CW], f32, name="rec", tag="abt")
            V.reciprocal(rec[:], sq[:])
            a_t = srcp.tile([128, NCHUNK * CW], f32, name="a_t", tag="srcp")
            V.tensor_tensor(a_t[:], cov[:], rec[:], Alu.mult)
            b_t = srcp.tile([128, NCHUNK * CW], f32, name="b_t", tag="srcp")
            G.tensor_tensor(b_t[:], a_t[:], mI[:], Alu.mult)
            G.tensor_tensor(b_t[:], mp_[:], b_t[:], Alu.subtract)
            tap("a", a_t)
            tap("b", b_t)

            hba = boxes.tile([128, NCHUNK * CW], f32, name="hba", tag="boxes")
            hbox(hba, a_t, cum, V)
            hbb = boxes.tile([128, NCHUNK * CW], f32, name="hbb", tag="boxes")
            hbox(hbb, b_t, cum, G)
            mean_a = boxes.tile([128, NCHUNK * CW], f32, name="mean_a", tag="boxes")
            vbox(mean_a, hba, vcum, hi_t, lo_t, V)
            mean_b = boxes.tile([128, NCHUNK * CW], f32, name="mean_b", tag="boxes")
            vbox(mean_b, hbb, vcum, hi_t, lo_t, G)

            T_t = abt.tile([128, NCHUNK * CW], f32, name="T_t", tag="abt")
            V.tensor_tensor(T_t[:], mean_a[:], guid[:], Alu.mult)
            V.tensor_tensor(T_t[:], T_t[:], mean_b[:], Alu.add)
            rT = abt.tile([128, NCHUNK * CW], f32, name="rT", tag="abt")
            V.reciprocal(rT[:], T_t[:])
            tap("T", T_t)
            tap("rT", rT)

            # ---- final: out_c = (0.5 x_c + (0.5 - A_c)) * rT + A_c ----
            for chn, xt in enumerate((xr, xg, xb)):
                d_t = abt.tile([128, NCHUNK * CW], f32, name=f"d{chn}", tag="dout", bufs=2)
                nc.scalar.activation(d_t[:], xt[:], Act.Identity,
                                     bias=chsc[:, 6 + chn:7 + chn], scale=0.5)
                V.tensor_tensor(d_t[:], d_t[:], rT[:], Alu.mult)
                V.tensor_scalar(d_t[:], d_t[:], chsc[:, chn:chn + 1], 0.0,
                                Alu.add, Alu.add)
                for c in range(NCHUNK):
                    nc.sync.dma_start(out=y_ext[s, chn, c * 128:(c + 1) * 128, :],
                                      in_=cview(d_t)[:, c, :])

    nc.compile()
    return nc


def _get_program():
    if "nc" not in _CACHE:
        _CACHE["nc"] = _build()
    return _CACHE["nc"]


def kernel(x: np.ndarray) -> np.ndarray:
    from concourse.bass_utils import run_bass_kernel_spmd
    x = np.ascontiguousarray(np.asarray(x, dtype=np.float32))
    assert x.shape == (16, 3, H, W), x.shape
    nc = _get_program()
    consts = _host_consts()
    in_maps = [{"x": x[2 * i:2 * i + 2], **consts} for i in range(8)]
    res = run_bass_kernel_spmd(nc, in_maps, list(range(8)))
    out = np.concatenate([res.results[i]["y"] for i in range(8)], axis=0)
    return out.astype(np.float32)
